# revision 11
# baseline (speedup 1.0000x reference)
"""CCNF RK4 sampling kernel for 8 Trainium2 NeuronCores.

Data-parallel: batch 2048 -> 256 per core, weights replicated.
On-core layout: features on partitions, batch on the free dim (N=256).
Matmuls run in float32r (fp32 data, fast PE mode).
"""

import os

import numpy as np

N_CORES = 8


def _build_program(theta0, context, W1, b1, W2, b2, W3, b3, n_steps):
    import concourse.bass as bass
    import concourse.mybir as mybir
    import concourse.tile as tile
    from concourse import bacc
    from concourse.bass_utils import run_bass_kernel_spmd

    f32 = mybir.dt.float32
    f32r = mybir.dt.float32r
    ALU = mybir.AluOpType
    SIGMOID = mybir.ActivationFunctionType.Sigmoid

    B, D = theta0.shape          # 2048, 32
    C = context.shape[1]         # 128
    IN, H2 = W1.shape            # 161, 1024
    H = W2.shape[0]              # 512
    assert H2 == 2 * H and W2.shape[1] == 2 * H and W3.shape == (H, D)
    assert IN == D + 1 + C
    assert B % N_CORES == 0
    Bs = B // N_CORES            # 256 per core
    steps = int(n_steps)
    dt = 1.0 / steps

    KC = H // 128                # 4 k-chunks for layer 2/3
    MJ = H // 128                # 4 output column-chunks per GLU half
    # layer-1 K split: rows [0:128) = theta(32)+t(1)+ctx[0:95); rows [128:161) = ctx[95:128)
    K1A = 128
    K1B = IN - K1A               # 33
    CA = K1A - (D + 1)           # 95 ctx features in chunk A

    # ---- host-side layout prep (shared across cores) ----
    W1 = np.ascontiguousarray(W1, np.float32)
    w1c1_h = np.ascontiguousarray(W1[:K1A])                    # [128, 1024]
    w1c2_h = np.ascontiguousarray(W1[K1A:])                    # [33, 1024]
    # [512,1024] -> [128, 4*1024]: chunk kc at cols kc*1024
    w2_h = np.ascontiguousarray(
        np.asarray(W2, np.float32).reshape(KC, 128, 2 * H).transpose(1, 0, 2).reshape(128, KC * 2 * H)
    )
    # [512,32] -> [128, 4*32]
    w3_h = np.ascontiguousarray(
        np.asarray(W3, np.float32).reshape(KC, 128, D).transpose(1, 0, 2).reshape(128, KC * D)
    )
    b1 = np.asarray(b1, np.float32)
    b2 = np.asarray(b2, np.float32)
    b1a_h = np.ascontiguousarray(b1[:H].reshape(MJ, 128).T)    # [128, 4] col j = chunk j
    b1b_h = np.ascontiguousarray(b1[H:].reshape(MJ, 128).T)
    b2a_h = np.ascontiguousarray(b2[:H].reshape(MJ, 128).T)
    b2b_h = np.ascontiguousarray(b2[H:].reshape(MJ, 128).T)
    b3r_h = np.ascontiguousarray(np.asarray(b3, np.float32).reshape(1, D))

    # ---- build the bass program (same program on all 8 cores) ----
    nc = bacc.Bacc("TRN2", target_bir_lowering=False)

    d_x1i = nc.dram_tensor("x1i", [K1A, Bs], f32r, kind="ExternalInput")
    d_x2i = nc.dram_tensor("x2i", [K1B, Bs], f32r, kind="ExternalInput")
    d_th0 = nc.dram_tensor("th0", [D, Bs], f32r, kind="ExternalInput")
    d_w1c1 = nc.dram_tensor("w1c1", [K1A, 2 * H], f32r, kind="ExternalInput")
    d_w1c2 = nc.dram_tensor("w1c2", [K1B, 2 * H], f32r, kind="ExternalInput")
    d_w2 = nc.dram_tensor("w2", [128, KC * 2 * H], f32r, kind="ExternalInput")
    d_w3 = nc.dram_tensor("w3", [128, KC * D], f32r, kind="ExternalInput")
    d_b1a = nc.dram_tensor("b1a", [128, MJ], f32, kind="ExternalInput")
    d_b1b = nc.dram_tensor("b1b", [128, MJ], f32, kind="ExternalInput")
    d_b2a = nc.dram_tensor("b2a", [128, MJ], f32, kind="ExternalInput")
    d_b2b = nc.dram_tensor("b2b", [128, MJ], f32, kind="ExternalInput")
    d_b3r = nc.dram_tensor("b3r", [1, D], f32r, kind="ExternalInput")
    d_out = nc.dram_tensor("out", [D, Bs], f32, kind="ExternalOutput")

    # RK4 coefficients: arg scale (for next eval's input), acc scale
    c_arg = [0.5 * dt, 0.5 * dt, dt]
    a_acc = [dt / 6.0, dt / 3.0, dt / 3.0, dt / 6.0]

    with tile.TileContext(nc) as tc:
        with (
            tc.tile_pool(name="const", bufs=1) as cpool,
            tc.tile_pool(name="psmm", bufs=7, space="PSUM") as ps_pool,
            tc.tile_pool(name="ps3", bufs=1, space="PSUM") as ps3_pool,
            tc.tile_pool(name="sig", bufs=6) as sig_pool,
            tc.tile_pool(name="hact", bufs=12) as h_pool,
            tc.tile_pool(name="accp", bufs=4) as acc_pool,
        ):
            tw1c1 = cpool.tile([K1A, 2 * H], f32r)
            tw1c2 = cpool.tile([K1B, 2 * H], f32r)
            tw2 = cpool.tile([128, KC * 2 * H], f32r)
            tw3 = cpool.tile([128, KC * D], f32r)
            tb1a = cpool.tile([128, MJ], f32)
            tb1b = cpool.tile([128, MJ], f32)
            tb2a = cpool.tile([128, MJ], f32)
            tb2b = cpool.tile([128, MJ], f32)
            tb3 = cpool.tile([1, D], f32r)
            tones = cpool.tile([1, Bs], f32r)
            tx1 = cpool.tile([K1A, Bs], f32r)   # rows: [theta(32) | t(1) | ctx[0:95)]
            tx2 = cpool.tile([K1B, Bs], f32r)   # ctx[95:128)
            tth0 = cpool.tile([D, Bs], f32r)    # initial theta

            nc.gpsimd.dma_start(tw1c1[:], d_w1c1[:])
            nc.gpsimd.dma_start(tw1c2[:], d_w1c2[:])
            nc.gpsimd.dma_start(tw2[:], d_w2[:])
            nc.gpsimd.dma_start(tw3[:], d_w3[:])
            nc.gpsimd.dma_start(tb1a[:], d_b1a[:])
            nc.gpsimd.dma_start(tb1b[:], d_b1b[:])
            nc.gpsimd.dma_start(tb2a[:], d_b2a[:])
            nc.gpsimd.dma_start(tb2b[:], d_b2b[:])
            nc.gpsimd.dma_start(tb3[:], d_b3r[:])
            nc.gpsimd.dma_start(tx1[:], d_x1i[:])
            nc.gpsimd.dma_start(tx2[:], d_x2i[:])
            nc.gpsimd.dma_start(tth0[:], d_th0[:])
            nc.gpsimd.memset(tones[:].bitcast(f32), 1.0)

            def mm(out_ap, lhsT_ap, rhs_ap, start, stop):
                nc.tensor.matmul(out_ap, lhsT_ap, rhs_ap, start=start, stop=stop)

            th_cur = tth0       # theta at start of current step
            t_written = 0.0     # t-row was set before the barrier

            for s in range(steps):
                for e in range(4):
                    t_val = (s + (0.0, 0.5, 0.5, 1.0)[e]) * dt
                    if t_val != t_written:
                        nc.gpsimd.memset(tx1[D : D + 1, :].bitcast(f32), float(t_val))
                        t_written = t_val

                    # ---- layer 1: [161]->1024, GLU -> h1 [512] ----
                    h1 = []
                    for j in range(MJ):
                        ps = ps_pool.tile([128, 2 * Bs], f32, tag="psmm")
                        for half, mj in ((0, j), (1, j + MJ)):
                            dst = ps[:, half * Bs : (half + 1) * Bs]
                            msl = slice(mj * 128, (mj + 1) * 128)
                            mm(dst, tw1c1[:, msl], tx1[:], start=True, stop=False)
                            mm(dst, tw1c2[:, msl], tx2[:], start=False, stop=True)
                        sg = sig_pool.tile([128, Bs], f32, tag="sig")
                        nc.scalar.activation(
                            sg[:], ps[:, Bs : 2 * Bs], SIGMOID, bias=tb1b[:, j : j + 1]
                        )
                        ht = h_pool.tile([128, Bs], f32r, tag="hact")
                        nc.vector.scalar_tensor_tensor(
                            ht[:], ps[:, 0:Bs], tb1a[:, j : j + 1], sg[:],
                            ALU.add, ALU.mult,
                        )
                        h1.append(ht)

                    # ---- layer 2: [512]->1024, GLU -> h2 [512] ----
                    h2 = []
                    for j in range(MJ):
                        ps = ps_pool.tile([128, 2 * Bs], f32, tag="psmm")
                        for half, mj in ((0, j), (1, j + MJ)):
                            dst = ps[:, half * Bs : (half + 1) * Bs]
                            for kc in range(KC):
                                csl = slice(kc * 2 * H + mj * 128, kc * 2 * H + (mj + 1) * 128)
                                mm(dst, tw2[:, csl], h1[kc][:],
                                   start=(kc == 0), stop=(kc == KC - 1))
                        sg = sig_pool.tile([128, Bs], f32, tag="sig")
                        nc.scalar.activation(
                            sg[:], ps[:, Bs : 2 * Bs], SIGMOID, bias=tb2b[:, j : j + 1]
                        )
                        ht = h_pool.tile([128, Bs], f32r, tag="hact")
                        nc.vector.scalar_tensor_tensor(
                            ht[:], ps[:, 0:Bs], tb2a[:, j : j + 1], sg[:],
                            ALU.add, ALU.mult,
                        )
                        h2.append(ht)

                    # ---- layer 3: [512]->32, k = h2 @ W3 + b3 (in PSUM) ----
                    ps3 = ps3_pool.tile([D, Bs], f32, tag="ps3")
                    for kc in range(KC):
                        mm(ps3[:], tw3[:, kc * D : (kc + 1) * D], h2[kc][:],
                           start=(kc == 0), stop=False)
                    mm(ps3[:], tb3[:], tones[:], start=False, stop=True)

                    # ---- RK4 bookkeeping ----
                    if e < 3:
                        # next eval's theta-argument: arg = c*k + theta0_of_step
                        nc.vector.scalar_tensor_tensor(
                            tx1[0:D, :], ps3[:], float(c_arg[e]), th_cur[:],
                            ALU.mult, ALU.add,
                        )
                    acc_new = acc_pool.tile([D, Bs], f32, tag="accp")
                    base = th_cur if e == 0 else acc_prev
                    nc.vector.scalar_tensor_tensor(
                        acc_new[:], ps3[:], float(a_acc[e]), base[:],
                        ALU.mult, ALU.add,
                    )
                    acc_prev = acc_new

                th_cur = acc_prev  # theta_{s+1} = theta_s + sum a_e k_e
                if s != steps - 1:
                    nc.gpsimd.tensor_copy(tx1[0:D, :], th_cur[:])

            nc.sync.dma_start(d_out[:], th_cur[:])

    # ---- per-core input maps ----
    in_maps = []
    for c in range(N_CORES):
        sl = slice(c * Bs, (c + 1) * Bs)
        th_T = np.ascontiguousarray(np.asarray(theta0[sl], np.float32).T)
        ctx_T = np.ascontiguousarray(np.asarray(context[sl], np.float32).T)
        x1i = np.concatenate(
            [th_T, np.zeros((1, Bs), np.float32), ctx_T[:CA]], axis=0
        )
        in_maps.append(
            {
                "x1i": np.ascontiguousarray(x1i),
                "x2i": np.ascontiguousarray(ctx_T[CA:]),
                "th0": th_T,
                "w1c1": w1c1_h,
                "w1c2": w1c2_h,
                "w2": w2_h,
                "w3": w3_h,
                "b1a": b1a_h,
                "b1b": b1b_h,
                "b2a": b2a_h,
                "b2b": b2b_h,
                "b3r": b3r_h,
            }
        )

    return nc, in_maps


def _build_and_run(theta0, context, W1, b1, W2, b2, W3, b3, n_steps):
    from concourse.bass_utils import run_bass_kernel_spmd

    nc, in_maps = _build_program(theta0, context, W1, b1, W2, b2, W3, b3, n_steps)
    nc.finalize()  # Bacc: split multi-sem waits + allocate registers
    res = run_bass_kernel_spmd(
        nc,
        in_maps,
        core_ids=list(range(N_CORES)),
        trace=bool(int(os.environ.get("KERNEL_TRACE", "0"))),
    )
    _build_and_run.last_results = res

    out = np.concatenate([r["out"].T for r in res.results], axis=0)
    return np.ascontiguousarray(out.astype(np.float32))


def kernel(theta0, context, W1, b1, W2, b2, W3, b3, n_steps):
    return _build_and_run(
        np.asarray(theta0), np.asarray(context), W1, b1, W2, b2, W3, b3, n_steps
    )
